# revision 11
# baseline (speedup 1.0000x reference)
"""Trainium2 Bass kernel for AIFSLocationAwareAttention (B=2,S=2048,D=1024,L=64,H=8).

Sharding: 8 cores; core c handles batch c//4, query rows [qoff, qoff+512) with
qoff=(c%4)*512, all 8 heads locally.  Token (key) order is rotated per-core so
the core's own queries are always tokens 0:512 -> the SPMD program is uniform.
No collectives.  Outputs are per-core disjoint slices, reassembled on host.

Math notes:
 - loc_bias (Wb/bb) adds a per-query constant along the softmax axis -> no-op.
 - softmax computed without max subtraction (logits are ~N(0,0.4), safe).
 - per-head normalization folded as 1/(8*s); Wo pre-scaled by 8 on host.
"""

import sys

sys.path.insert(0, "/opt/trn_rl_repo")

import numpy as np
import ml_dtypes

import concourse.bass as bass  # noqa: F401  (registers engines)
import concourse.tile as tile
from concourse import bacc, mybir
from concourse.bass_utils import run_bass_kernel_spmd

P = 128
B, S, D, L, H = 2, 2048, 1024, 64, 8
DH = D // H           # 128
QS = 512              # queries per core
NKC = S // P          # 16 key chunks
ND = D // P           # 8 feature chunks
NQB = QS // 512       # 1 (q fits one matmul free dim)
LN_EPS = 1e-5
SCALE = 1.0 / np.sqrt(np.float32(DH))

F32 = mybir.dt.float32
BF16 = mybir.dt.bfloat16
AF = mybir.ActivationFunctionType
ALU = mybir.AluOpType


def build_program():
    nc = bacc.Bacc(None, target_bir_lowering=False)

    # ---- per-core DRAM inputs (shapes uniform across cores) ----
    featT_bf = nc.dram_tensor("featT_bf", [D, S], BF16, kind="ExternalInput")
    featq = nc.dram_tensor("featq", [D, QS], F32, kind="ExternalInput")
    locTe = nc.dram_tensor("locTe", [L + 1, S], BF16, kind="ExternalInput")
    wlocTe = nc.dram_tensor("wlocTe", [L + 1, D], BF16, kind="ExternalInput")
    wqT = nc.dram_tensor("wqT", [D, D], BF16, kind="ExternalInput")
    wkT = nc.dram_tensor("wkT", [D, D], BF16, kind="ExternalInput")
    wvT = nc.dram_tensor("wvT", [D, D], BF16, kind="ExternalInput")
    wo8T = nc.dram_tensor("wo8T", [D, D], BF16, kind="ExternalInput")
    bq_pc = nc.dram_tensor("bq_pc", [P, ND], F32, kind="ExternalInput")
    bk_pc = nc.dram_tensor("bk_pc", [P, ND], F32, kind="ExternalInput")
    bv_row = nc.dram_tensor("bv_row", [1, D], BF16, kind="ExternalInput")
    bo_pc = nc.dram_tensor("bo_pc", [P, ND], F32, kind="ExternalInput")
    gamma_pc = nc.dram_tensor("gamma_pc", [P, ND], F32, kind="ExternalInput")
    beta_pc = nc.dram_tensor("beta_pc", [P, ND], F32, kind="ExternalInput")

    yT = nc.dram_tensor("yT", [D, QS], F32, kind="ExternalOutput")
    avgT = nc.dram_tensor("avgT", [S, QS], BF16, kind="ExternalOutput")

    with tile.TileContext(nc) as tc:
        # ---------- persistent SBUF ----------
        persist = tc.alloc_tile_pool(name="persist", bufs=1)
        KT_sb = persist.tile([P, ND, S], BF16, tag="KT")        # [dh, h, k]
        V_sb = persist.tile([P, NKC, D], BF16, tag="V")         # [k_in, kc, d]
        QT_sb = persist.tile([P, ND, QS], BF16, tag="QT")       # [dh, h, q]
        attn_sb = persist.tile([P, ND, QS], BF16, tag="attn")   # [dh, h, q]
        acc_sb = persist.tile([P, NKC, QS], BF16, tag="acc")    # avg accum [k_in, kc, q]

        small = tc.alloc_tile_pool(name="small", bufs=1)
        bcp = tc.alloc_tile_pool(name="bcast", bufs=2)
        invp = tc.alloc_tile_pool(name="invs", bufs=2)
        bq_sb = small.tile([P, ND], F32, tag="bq")
        bk_sb = small.tile([P, ND], F32, tag="bk")
        bo_sb = small.tile([P, ND], F32, tag="bo")
        gamma_sb = small.tile([P, ND], F32, tag="gamma")
        beta_sb = small.tile([P, ND], F32, tag="beta")
        bv_sb = small.tile([1, D], BF16, tag="bv")
        ones_bf_col = small.tile([P, 1], BF16, tag="ones_bf")    # lhsT for key-sums
        ones_bf_row = small.tile([1, P], BF16, tag="onesr_bf")   # lhsT for V bias
        ones_f_col = small.tile([P, 1], F32, tag="ones_f")       # lhsT for LN sums
        ones_f_row = small.tile([1, P], F32, tag="onesr_f")      # lhsT for broadcasts
        eps_sb = small.tile([1, 1], F32, tag="eps")

        nc.sync.dma_start(bq_sb[:], bq_pc[:])
        nc.sync.dma_start(bk_sb[:], bk_pc[:])
        nc.sync.dma_start(bo_sb[:], bo_pc[:])
        nc.sync.dma_start(gamma_sb[:], gamma_pc[:])
        nc.sync.dma_start(beta_sb[:], beta_pc[:])
        nc.sync.dma_start(bv_sb[:], bv_row[:])
        nc.vector.memset(ones_bf_col[:], 1.0)
        nc.vector.memset(ones_bf_row[:], 1.0)
        nc.vector.memset(ones_f_col[:], 1.0)
        nc.vector.memset(ones_f_row[:], 1.0)
        nc.vector.memset(eps_sb[:], LN_EPS)

        # ---------- phase 1+2: combined = feat + loc@WlocT + bloc; K/V/Q ----------
        with (
            tc.tile_pool(name="p12", bufs=1) as p12,
            tc.tile_pool(name="featst", bufs=2) as featst,
            tc.tile_pool(name="wx", bufs=2) as wxp,
            tc.tile_pool(name="psum12", bufs=2, space="PSUM") as ps12,
        ):
            locTe_sb = p12.tile([L + 1, S], BF16, tag="locTe")
            wlocTe_sb = p12.tile([L + 1, D], BF16, tag="wlocTe")
            combT = p12.tile([P, ND, S], BF16, tag="combT")
            nc.sync.dma_start(locTe_sb[:], locTe[:])
            nc.sync.dma_start(wlocTe_sb[:], wlocTe[:])

            featT_r = featT_bf.rearrange("(c p) t -> c p t", p=P)
            for di in range(ND):
                fchunk = featst.tile([P, S], BF16, tag="fchunk")
                nc.sync.dma_start(fchunk[:], featT_r[di])
                ps = ps12.tile([P, S], F32, tag="ps_big")
                for nb in range(S // 512):
                    nc.tensor.matmul(
                        ps[:, nb * 512:(nb + 1) * 512],
                        wlocTe_sb[:, di * P:(di + 1) * P],
                        locTe_sb[:, nb * 512:(nb + 1) * 512],
                        start=True, stop=True,
                    )
                nc.vector.tensor_add(combT[:, di, :], ps[:], fchunk[:])

            # --- K projection: KT[dh, h, k] (+bk via ACT bias) ---
            wk_sb = wxp.tile([P, ND, D], BF16, tag="wx")
            nc.sync.dma_start(wk_sb[:], wkT.rearrange("(s p) d -> p s d", p=P))
            for do in range(ND):
                ps = ps12.tile([P, S], F32, tag="ps_big")
                for st in range(ND):
                    for nb in range(S // 512):
                        nc.tensor.matmul(
                            ps[:, nb * 512:(nb + 1) * 512],
                            wk_sb[:, st, do * P:(do + 1) * P],
                            combT[:, st, nb * 512:(nb + 1) * 512],
                            start=(st == 0), stop=(st == ND - 1),
                        )
                nc.scalar.activation(KT_sb[:, do, :], ps[:], AF.Identity,
                                     bias=bk_sb[:, do:do + 1], scale=1.0)

            # --- Q projection (own queries = tokens 0:QS) ---
            wq_sb = wxp.tile([P, ND, D], BF16, tag="wx")
            nc.sync.dma_start(wq_sb[:], wqT.rearrange("(s p) d -> p s d", p=P))
            for do in range(ND):
                ps = ps12.tile([P, QS], F32, tag="ps_big")
                for st in range(ND):
                    nc.tensor.matmul(
                        ps[:, :],
                        wq_sb[:, st, do * P:(do + 1) * P],
                        combT[:, st, :QS],
                        start=(st == 0), stop=(st == ND - 1),
                    )
                nc.scalar.activation(QT_sb[:, do, :], ps[:], AF.Identity,
                                     bias=bq_sb[:, do:do + 1], scale=1.0)

            # --- V projection: V[k_in, kc, d] (+bv via ones-row matmul) ---
            wv_sb = wxp.tile([P, ND, D], BF16, tag="wx")
            nc.sync.dma_start(wv_sb[:], wvT.rearrange("(s p) d -> p s d", p=P))
            for kc in range(NKC):
                ps = ps12.tile([P, D], F32, tag="ps_big")
                for st in range(ND):
                    for nb in range(D // 512):
                        nc.tensor.matmul(
                            ps[:, nb * 512:(nb + 1) * 512],
                            combT[:, st, kc * P:(kc + 1) * P],
                            wv_sb[:, st, nb * 512:(nb + 1) * 512],
                            start=(st == 0), stop=False,
                        )
                for nb in range(D // 512):
                    nc.tensor.matmul(
                        ps[:, nb * 512:(nb + 1) * 512],
                        ones_bf_row[:, :],
                        bv_sb[:, nb * 512:(nb + 1) * 512],
                        start=False, stop=True,
                    )
                nc.vector.tensor_copy(V_sb[:, kc, :], ps[:])

        # ---------- phase 3: attention per head ----------
        with (
            tc.tile_pool(name="ew", bufs=2) as ewp,
            tc.tile_pool(name="ps_sc", bufs=2, space="PSUM") as ps_sc,
            tc.tile_pool(name="ps_att", bufs=2, space="PSUM") as ps_att,
            tc.tile_pool(name="ps_sum", bufs=1, space="PSUM") as ps_sum,
        ):
            for h in range(H):
                ew = ewp.tile([P, NKC, QS], BF16, tag="ew")
                sums = ps_sum.tile([1, QS], F32, tag="sums")
                att = ps_att.tile([P, QS], F32, tag="attps")
                for sg in range(NKC // 2):
                    sc = ps_sc.tile([P, 2, QS], F32, tag="sc")
                    for j in range(2):
                        c = 2 * sg + j
                        nc.tensor.matmul(
                            sc[:, j, :],
                            KT_sb[:, h, c * P:(c + 1) * P],
                            QT_sb[:, h, :],
                            start=True, stop=True,
                        )
                    nc.scalar.activation(ew[:, 2 * sg:2 * sg + 2, :], sc[:],
                                         AF.Exp, scale=float(SCALE))
                    for j in range(2):
                        c = 2 * sg + j
                        nc.tensor.matmul(
                            sums[:, :], ones_bf_col[:, :], ew[:, c, :],
                            start=(c == 0), stop=(c == NKC - 1),
                        )
                        nc.tensor.matmul(
                            att[:, :], V_sb[:, c, h * P:(h + 1) * P], ew[:, c, :],
                            start=(c == 0), stop=(c == NKC - 1),
                        )
                # inv_s, broadcast over partitions via PE, then *0.125 -> bf16
                inv_s = invp.tile([1, QS], F32, tag="invs")
                nc.vector.reciprocal(inv_s[:], sums[:])
                bc_ps = ps_att.tile([P, QS], F32, tag="attps")
                nc.tensor.matmul(bc_ps[:], ones_f_row[:, :], inv_s[:],
                                 start=True, stop=True)
                bc8 = bcp.tile([P, QS], BF16, tag="bc8")
                nc.vector.tensor_scalar_mul(bc8[:], bc_ps[:], 0.125)
                # attended (normalized by 1/(8s); Wo is pre-scaled by 8)
                nc.vector.tensor_mul(attn_sb[:, h, :], att[:], bc8[:])
                # weights*(1/(8s)) accumulated into the head-mean
                nc.vector.tensor_tensor(
                    ew[:], ew[:],
                    bc8[:, None, :].to_broadcast([P, NKC, QS]),
                    ALU.mult,
                )
                if h == 0:
                    nc.vector.tensor_copy(acc_sb[:], ew[:])
                else:
                    nc.vector.tensor_add(acc_sb[:], acc_sb[:], ew[:])

        nc.sync.dma_start(avgT.rearrange("(c p) q -> p c q", p=P), acc_sb[:])

        # ---------- phase 4: out-proj + residual + LayerNorm ----------
        with (
            tc.tile_pool(name="ln", bufs=1) as lnp,
            tc.tile_pool(name="lnsq", bufs=1) as lnsq,
            tc.tile_pool(name="wo", bufs=1) as wop,
            tc.tile_pool(name="ps_o", bufs=3, space="PSUM") as ps_o,
            tc.tile_pool(name="ps_s", bufs=2, space="PSUM") as ps_s,
        ):
            wo_sb = wop.tile([P, ND, D], BF16, tag="wo")
            nc.sync.dma_start(wo_sb[:], wo8T.rearrange("(s p) d -> p s d", p=P))
            fq_sb = lnsq.tile([P, ND, QS], F32, tag="fq")
            nc.sync.dma_start(fq_sb[:], featq.rearrange("(c p) q -> p c q", p=P))
            resid = lnp.tile([P, ND, QS], F32, tag="resid")
            mu_ps = ps_s.tile([1, QS], F32, tag="stat")
            for do in range(ND):
                ps = ps_o.tile([P, QS], F32, tag="pso")
                for st in range(ND):
                    nc.tensor.matmul(
                        ps[:, :],
                        wo_sb[:, st, do * P:(do + 1) * P],
                        attn_sb[:, st, :],
                        start=(st == 0), stop=(st == ND - 1),
                    )
                # resid = (ps + bo) + featq
                nc.vector.scalar_tensor_tensor(
                    resid[:, do, :], ps[:], bo_sb[:, do:do + 1], fq_sb[:, do, :],
                    ALU.add, ALU.add,
                )
                nc.tensor.matmul(mu_ps[:], ones_f_col[:, :], resid[:, do, :],
                                 start=(do == 0), stop=(do == ND - 1))
            sq = lnsq.tile([P, ND, QS], F32, tag="fq")
            nc.scalar.activation(sq[:], resid[:], AF.Square)
            x2_ps = ps_s.tile([1, QS], F32, tag="stat")
            for do in range(ND):
                nc.tensor.matmul(x2_ps[:], ones_f_col[:, :], sq[:, do, :],
                                 start=(do == 0), stop=(do == ND - 1))
            # scalars: mu, var, rstd  (all [1, QS])
            mu = small.tile([1, QS], F32, tag="mu")
            m2 = small.tile([1, QS], F32, tag="m2")
            var = small.tile([1, QS], F32, tag="var")
            rvar = small.tile([1, QS], F32, tag="rvar")
            rstd = small.tile([1, QS], F32, tag="rstd")
            nc.vector.tensor_scalar_mul(mu[:], mu_ps[:], 1.0 / D)
            nc.vector.tensor_scalar_mul(m2[:], x2_ps[:], 1.0 / D)
            nc.vector.tensor_tensor(var[:], mu[:], mu[:], ALU.mult)
            nc.vector.scalar_tensor_tensor(var[:], m2[:], LN_EPS, var[:],
                                           ALU.add, ALU.subtract)
            nc.vector.reciprocal(rvar[:], var[:])
            nc.scalar.sqrt(rstd[:], rvar[:])
            # broadcast mu and rstd to [P, QS]
            bmu_ps = ps_o.tile([P, QS], F32, tag="pso")
            nc.tensor.matmul(bmu_ps[:], ones_f_row[:, :], mu[:],
                             start=True, stop=True)
            brs_ps = ps_o.tile([P, QS], F32, tag="pso")
            nc.tensor.matmul(brs_ps[:], ones_f_row[:, :], rstd[:],
                             start=True, stop=True)
            bmu = small.tile([P, QS], F32, tag="bmu")
            brs = small.tile([P, QS], F32, tag="brs")
            nc.vector.tensor_copy(bmu[:], bmu_ps[:])
            nc.vector.tensor_copy(brs[:], brs_ps[:])
            yT_r = yT.rearrange("(c p) q -> p c q", p=P)
            ysb = lnp.tile([P, ND, QS], F32, tag="ysb")
            for do in range(ND):
                t = lnp.tile([P, QS], F32, tag="t")
                nc.vector.tensor_sub(t[:], resid[:, do, :], bmu[:])
                nc.vector.tensor_mul(t[:], t[:], brs[:])
                nc.vector.tensor_scalar(
                    ysb[:, do, :], t[:],
                    gamma_sb[:, do:do + 1], beta_sb[:, do:do + 1],
                    ALU.mult, ALU.add,
                )
            nc.sync.dma_start(yT_r[:], ysb[:])

        invp.release()
        bcp.release()
        small.release()
        persist.release()

    nc.compile()
    return nc


_NC_CACHE = None


def _get_program():
    global _NC_CACHE
    if _NC_CACHE is None:
        _NC_CACHE = build_program()
    return _NC_CACHE


def make_in_maps(features, locations, Wloc, bloc, Wq, bq, Wk, bk, Wv, bv,
                 Wb, bb, Wo, bo, gamma, beta):
    bf = ml_dtypes.bfloat16
    wlocTe = np.concatenate([Wloc.T, bloc[None, :]], axis=0).astype(bf)
    wqT = Wq.T.astype(bf)
    wkT = Wk.T.astype(bf)
    wvT = Wv.T.astype(bf)
    wo8T = (8.0 * Wo).T.astype(bf)
    bq_pc = np.ascontiguousarray(bq.reshape(ND, P).T.astype(np.float32))
    bk_pc = np.ascontiguousarray(bk.reshape(ND, P).T.astype(np.float32))
    bv_row = bv[None, :].astype(bf)
    bo_pc = np.ascontiguousarray(bo.reshape(ND, P).T.astype(np.float32))
    gamma_pc = np.ascontiguousarray(gamma.reshape(ND, P).T.astype(np.float32))
    beta_pc = np.ascontiguousarray(beta.reshape(ND, P).T.astype(np.float32))
    in_maps = []
    for c in range(8):
        b = c // 4
        qoff = (c % 4) * QS
        perm = (qoff + np.arange(S)) % S
        feat_rot = features[b][perm]          # (S, D)
        loc_rot = locations[b][perm]          # (S, L)
        featT_bf = np.ascontiguousarray(feat_rot.T).astype(bf)
        featq = np.ascontiguousarray(feat_rot[:QS].T.astype(np.float32))
        locTe = np.concatenate(
            [loc_rot.T, np.ones((1, S), np.float32)], axis=0).astype(bf)
        in_maps.append({
            "featT_bf": featT_bf, "featq": featq, "locTe": locTe,
            "wlocTe": wlocTe, "wqT": wqT, "wkT": wkT, "wvT": wvT,
            "wo8T": wo8T, "bq_pc": bq_pc, "bk_pc": bk_pc, "bv_row": bv_row,
            "bo_pc": bo_pc, "gamma_pc": gamma_pc, "beta_pc": beta_pc,
        })
    return in_maps


def unshard(results):
    y = np.empty((B, S, D), np.float32)
    avg = np.empty((B, S, S), np.float32)
    for c in range(8):
        b = c // 4
        qoff = (c % 4) * QS
        yT_out = np.asarray(results[c]["yT"], dtype=np.float32)       # (D, QS)
        avgT_out = np.asarray(results[c]["avgT"]).astype(np.float32)  # (S, QS)
        y[b, qoff:qoff + QS, :] = yT_out.T
        avg[b, qoff:qoff + QS, :] = np.roll(avgT_out, qoff, axis=0).T
    return y, avg


def kernel(**inputs):
    inputs = {k: np.asarray(v) for k, v in inputs.items()}
    nc = _get_program()
    in_maps = make_in_maps(**inputs)
    res = run_bass_kernel_spmd(nc, in_maps, core_ids=list(range(8)))
    return unshard(res.results)


# revision 20
# speedup vs baseline: 1.8004x; 1.8004x over previous
"""Trainium2 Bass kernel for AIFSLocationAwareAttention (B=2,S=2048,D=1024,L=64,H=8).

Sharding: 8 cores; core c handles batch c//4, query rows [qoff, qoff+512),
all 8 heads locally.  K/V are computed on the owning core's query shard and
AllGathered within the 4-core batch group, so every projection FLOP is done
exactly once.  Outputs are per-core disjoint slices, reassembled on host.

Math notes:
 - loc_bias (Wb/bb) adds a per-query constant along the softmax axis -> no-op
   under softmax; skipped entirely.
 - softmax computed without max subtraction (logits are ~N(0,0.4), safe).
 - per-head normalization folded as 1/(8*s); Wo pre-scaled by 8 on host, so
   one scaled exp() tensor serves both the attended matmul and the
   mean-over-heads output.
 - all matmul operands bf16 (PSUM accumulation stays f32).
"""

import sys

sys.path.insert(0, "/opt/trn_rl_repo")

import numpy as np
import ml_dtypes

import concourse.bass as bass  # noqa: F401
import concourse.tile as tile
from concourse import bacc, mybir
from concourse.bass_utils import run_bass_kernel_spmd

P = 128
B, S, D, L, H = 2, 2048, 1024, 64, 8
DH = D // H           # 128
QS = 512              # queries per core
NKC = S // P          # 16 key chunks
ND = D // P           # 8 feature chunks
NQC = QS // P         # 4 own key chunks
GW = 4                # allgather group width
LN_EPS = 1e-5
SCALE = 1.0 / np.sqrt(np.float32(DH))

F32 = mybir.dt.float32
BF16 = mybir.dt.bfloat16
AF = mybir.ActivationFunctionType
ALU = mybir.AluOpType


def build_program():
    nc = bacc.Bacc(None, target_bir_lowering=False)

    # ---- per-core DRAM inputs (uniform shapes; data differs per core) ----
    featq = nc.dram_tensor("featq", [D, QS], F32, kind="ExternalInput")
    locTe = nc.dram_tensor("locTe", [L + 1, QS], BF16, kind="ExternalInput")
    wlocTe = nc.dram_tensor("wlocTe", [L + 1, D], BF16, kind="ExternalInput")
    wqT = nc.dram_tensor("wqT", [D, D], BF16, kind="ExternalInput")
    wkT = nc.dram_tensor("wkT", [D, D], BF16, kind="ExternalInput")
    wvT = nc.dram_tensor("wvT", [D, D], BF16, kind="ExternalInput")
    wo8T = nc.dram_tensor("wo8T", [D, D], BF16, kind="ExternalInput")
    bq_pc = nc.dram_tensor("bq_pc", [P, ND], F32, kind="ExternalInput")
    bk_pc = nc.dram_tensor("bk_pc", [P, ND], F32, kind="ExternalInput")
    bv_row = nc.dram_tensor("bv_row", [1, D], BF16, kind="ExternalInput")
    bo_pc = nc.dram_tensor("bo_pc", [P, ND], F32, kind="ExternalInput")
    gamma_pc = nc.dram_tensor("gamma_pc", [P, ND], F32, kind="ExternalInput")
    beta_pc = nc.dram_tensor("beta_pc", [P, ND], F32, kind="ExternalInput")

    yT = nc.dram_tensor("yT", [D, QS], F32, kind="ExternalOutput")
    avgT = nc.dram_tensor("avgT", [S, QS], BF16, kind="ExternalOutput")

    # K/V allgather bounce buffers (internal DRAM)
    # block 0: K shard as (D, QS) row-major; block 1: V shard as (QS, D)
    kvag_in = nc.dram_tensor("kvag_in", [2, D * QS], BF16)
    kvag_out = nc.dram_tensor("kvag_out", [GW, 2, D * QS], BF16)

    with tile.TileContext(nc) as tc:
        # ---------- persistent SBUF ----------
        persist = tc.alloc_tile_pool(name="persist", bufs=1)
        KT_sb = persist.tile([P, ND, S], BF16, tag="KT")        # [dh, h, k]
        V_sb = persist.tile([P, NKC, D], BF16, tag="V")         # [k_in, kc, d]
        QT_sb = persist.tile([P, ND, QS], BF16, tag="QT")       # [dh, h, q]
        attn_sb = persist.tile([P, ND, QS], BF16, tag="attn")   # [dh, h, q]
        acc_sb = persist.tile([P, NKC, QS], BF16, tag="acc")    # avg accum
        fq_sb = persist.tile([P, ND, QS], F32, tag="fq")        # features (resid)

        small = tc.alloc_tile_pool(name="small", bufs=1)
        bcp = tc.alloc_tile_pool(name="bcast", bufs=2)
        invp = tc.alloc_tile_pool(name="invs", bufs=2)
        bq_sb = small.tile([P, ND], F32, tag="bq")
        bk_sb = small.tile([P, ND], F32, tag="bk")
        bo_sb = small.tile([P, ND], F32, tag="bo")
        gamma_sb = small.tile([P, ND], F32, tag="gamma")
        beta_sb = small.tile([P, ND], F32, tag="beta")
        bv_sb = small.tile([1, D], BF16, tag="bv")
        ones_bf_col = small.tile([P, 1], BF16, tag="ones_bf")    # lhsT for key-sums
        ones_bf_row = small.tile([1, P], BF16, tag="onesr_bf")   # lhsT for V bias
        ones_f_col = small.tile([P, 1], F32, tag="ones_f")       # lhsT for LN sums
        ones_f_row = small.tile([1, P], F32, tag="onesr_f")      # lhsT for broadcasts

        nc.sync.dma_start(bq_sb[:], bq_pc[:])
        nc.sync.dma_start(bk_sb[:], bk_pc[:])
        nc.sync.dma_start(bo_sb[:], bo_pc[:])
        nc.sync.dma_start(gamma_sb[:], gamma_pc[:])
        nc.sync.dma_start(beta_sb[:], beta_pc[:])
        nc.sync.dma_start(bv_sb[:], bv_row[:])
        nc.sync.dma_start(fq_sb[:], featq.rearrange("(c p) q -> p c q", p=P))
        nc.vector.memset(ones_bf_col[:], 1.0)
        nc.vector.memset(ones_bf_row[:], 1.0)
        nc.vector.memset(ones_f_col[:], 1.0)
        nc.vector.memset(ones_f_row[:], 1.0)

        # ---------- phase 1+2: combined; K/V shards; allgather; Q ----------
        with (
            tc.tile_pool(name="p12", bufs=1) as p12,
            tc.tile_pool(name="wx", bufs=2) as wxp,
            tc.tile_pool(name="psum12", bufs=3, space="PSUM") as ps12,
        ):
            locTe_sb = p12.tile([L + 1, QS], BF16, tag="locTe")
            wlocTe_sb = p12.tile([L + 1, D], BF16, tag="wlocTe")
            combT = p12.tile([P, ND, QS], BF16, tag="combT")
            nc.sync.dma_start(locTe_sb[:], locTe[:])
            nc.sync.dma_start(wlocTe_sb[:], wlocTe[:])

            for di in range(ND):
                ps = ps12.tile([P, D], F32, tag="ps_big")
                nc.tensor.matmul(
                    ps[:, :QS],
                    wlocTe_sb[:, di * P:(di + 1) * P],
                    locTe_sb[:, :],
                    start=True, stop=True,
                )
                nc.vector.tensor_add(combT[:, di, :], ps[:, :QS], fq_sb[:, di, :])

            # --- K shard: KTs[dh, h, own-q] (+bk via ACT bias) ---
            KTs = p12.tile([P, ND, QS], BF16, tag="KTs")
            wk_sb = wxp.tile([P, ND, D], BF16, tag="wx")
            nc.sync.dma_start(wk_sb[:], wkT.rearrange("(s p) d -> p s d", p=P))
            for do in range(ND):
                ps = ps12.tile([P, D], F32, tag="ps_big")
                for st in range(ND):
                    nc.tensor.matmul(
                        ps[:, :QS],
                        wk_sb[:, st, do * P:(do + 1) * P],
                        combT[:, st, :],
                        start=(st == 0), stop=(st == ND - 1),
                    )
                nc.scalar.activation(KTs[:, do, :], ps[:, :QS], AF.Identity,
                                     bias=bk_sb[:, do:do + 1], scale=1.0)
            nc.sync.dma_start(
                kvag_in[0].rearrange("(c p q) -> p c q", p=P, q=QS), KTs[:])

            # --- V shard: Vs[k_in, own-kc, d] (+bv via ones-row matmul) ---
            Vs = p12.tile([P, NQC, D], BF16, tag="Vs")
            wv_sb = wxp.tile([P, ND, D], BF16, tag="wx")
            nc.sync.dma_start(wv_sb[:], wvT.rearrange("(s p) d -> p s d", p=P))
            for kc in range(NQC):
                ps = ps12.tile([P, D], F32, tag="ps_big")
                for st in range(ND):
                    for nb in range(D // 512):
                        nc.tensor.matmul(
                            ps[:, nb * 512:(nb + 1) * 512],
                            combT[:, st, kc * P:(kc + 1) * P],
                            wv_sb[:, st, nb * 512:(nb + 1) * 512],
                            start=(st == 0), stop=False,
                        )
                for nb in range(D // 512):
                    nc.tensor.matmul(
                        ps[:, nb * 512:(nb + 1) * 512],
                        ones_bf_row[:, :],
                        bv_sb[:, nb * 512:(nb + 1) * 512],
                        start=False, stop=True,
                    )
                nc.vector.tensor_copy(Vs[:, kc, :], ps[:])
            nc.sync.dma_start(
                kvag_in[1].rearrange("(c p d) -> p c d", p=P, d=D), Vs[:])

            # --- allgather K|V within the batch group ---
            nc.gpsimd.collective_compute(
                "AllGather",
                ALU.bypass,
                ins=[kvag_in[:]],
                outs=[kvag_out[:]],
                replica_groups=[[0, 1, 2, 3], [4, 5, 6, 7]],
            )

            # --- Q projection (overlaps the allgather) ---
            wq_sb = wxp.tile([P, ND, D], BF16, tag="wx")
            nc.sync.dma_start(wq_sb[:], wqT.rearrange("(s p) d -> p s d", p=P))
            for do in range(ND):
                ps = ps12.tile([P, D], F32, tag="ps_big")
                for st in range(ND):
                    nc.tensor.matmul(
                        ps[:, :QS],
                        wq_sb[:, st, do * P:(do + 1) * P],
                        combT[:, st, :],
                        start=(st == 0), stop=(st == ND - 1),
                    )
                nc.scalar.activation(QT_sb[:, do, :], ps[:, :QS], AF.Identity,
                                     bias=bq_sb[:, do:do + 1], scale=1.0)

            # --- unpack gathered K/V into full-sequence SBUF tiles ---
            agK = kvag_out[:, 0].rearrange("r (c p q) -> r c p q", p=P, q=QS)
            for h in range(ND):
                nc.sync.dma_start(
                    KT_sb[:, h, :].rearrange("p (r q) -> p r q", r=GW),
                    agK[:, h].rearrange("r p q -> p r q"),
                )
            for r in range(GW):
                nc.sync.dma_start(
                    V_sb[:, r * NQC:(r + 1) * NQC, :],
                    kvag_out[r, 1].rearrange("(k p d) -> p k d", p=P, d=D),
                )

        # ---------- phase 3: attention per head ----------
        with (
            tc.tile_pool(name="ew", bufs=2) as ewp,
            tc.tile_pool(name="ps_sc", bufs=2, space="PSUM") as ps_sc,
            tc.tile_pool(name="ps_att", bufs=2, space="PSUM") as ps_att,
            tc.tile_pool(name="ps_sum", bufs=1, space="PSUM") as ps_sum,
        ):
            for h in range(H):
                ew = ewp.tile([P, NKC, QS], BF16, tag="ew")
                sums = ps_sum.tile([1, QS], F32, tag="sums")
                att = ps_att.tile([P, QS], F32, tag="attps")
                for sg in range(NKC // 2):
                    sc = ps_sc.tile([P, 2, QS], F32, tag="sc")
                    for j in range(2):
                        c = 2 * sg + j
                        nc.tensor.matmul(
                            sc[:, j, :],
                            KT_sb[:, h, c * P:(c + 1) * P],
                            QT_sb[:, h, :],
                            start=True, stop=True,
                        )
                    nc.scalar.activation(ew[:, 2 * sg:2 * sg + 2, :], sc[:],
                                         AF.Exp, scale=float(SCALE))
                    for j in range(2):
                        c = 2 * sg + j
                        nc.tensor.matmul(
                            sums[:, :], ones_bf_col[:, :], ew[:, c, :],
                            start=(c == 0), stop=(c == NKC - 1),
                        )
                        nc.tensor.matmul(
                            att[:, :], V_sb[:, c, h * P:(h + 1) * P], ew[:, c, :],
                            start=(c == 0), stop=(c == NKC - 1),
                        )
                # inv_s broadcast over partitions via PE, then *1/8 -> bf16
                inv_s = invp.tile([1, QS], F32, tag="invs")
                nc.vector.reciprocal(inv_s[:], sums[:])
                bc_ps = ps_att.tile([P, QS], F32, tag="attps")
                nc.tensor.matmul(bc_ps[:], ones_f_row[:, :], inv_s[:],
                                 start=True, stop=True)
                bc8 = bcp.tile([P, QS], BF16, tag="bc8")
                nc.vector.tensor_scalar_mul(bc8[:], bc_ps[:], 0.125)
                # attended (normalized by 1/(8s); Wo is pre-scaled by 8)
                nc.vector.tensor_mul(attn_sb[:, h, :], att[:], bc8[:])
                # weights*(1/(8s)) accumulated into the head-mean
                nc.vector.tensor_tensor(
                    ew[:], ew[:],
                    bc8[:, None, :].to_broadcast([P, NKC, QS]),
                    ALU.mult,
                )
                if h == 0:
                    nc.vector.tensor_copy(acc_sb[:], ew[:])
                else:
                    nc.vector.tensor_add(acc_sb[:], acc_sb[:], ew[:])

        nc.sync.dma_start(avgT.rearrange("(c p) q -> p c q", p=P), acc_sb[:])

        # ---------- phase 4: out-proj + residual + LayerNorm ----------
        with (
            tc.tile_pool(name="ln", bufs=1) as lnp,
            tc.tile_pool(name="lnt", bufs=3) as lnt,
            tc.tile_pool(name="wo", bufs=1) as wop,
            tc.tile_pool(name="ps_o", bufs=3, space="PSUM") as ps_o,
            tc.tile_pool(name="ps_s", bufs=2, space="PSUM") as ps_s,
        ):
            wo_sb = wop.tile([P, ND, D], BF16, tag="wo")
            nc.sync.dma_start(wo_sb[:], wo8T.rearrange("(s p) d -> p s d", p=P))
            resid = lnp.tile([P, ND, QS], F32, tag="resid")
            mu_ps = ps_s.tile([1, QS], F32, tag="stat")
            x2_ps = ps_s.tile([1, QS], F32, tag="stat")
            for do in range(ND):
                ps = ps_o.tile([P, QS], F32, tag="pso")
                for st in range(ND):
                    nc.tensor.matmul(
                        ps[:, :],
                        wo_sb[:, st, do * P:(do + 1) * P],
                        attn_sb[:, st, :],
                        start=(st == 0), stop=(st == ND - 1),
                    )
                # resid = (ps + bo) + featq
                nc.vector.scalar_tensor_tensor(
                    resid[:, do, :], ps[:], bo_sb[:, do:do + 1], fq_sb[:, do, :],
                    ALU.add, ALU.add,
                )
                nc.tensor.matmul(mu_ps[:], ones_f_col[:, :], resid[:, do, :],
                                 start=(do == 0), stop=(do == ND - 1))
                sq = lnt.tile([P, QS], F32, tag="sq")
                nc.scalar.activation(sq[:], resid[:, do, :], AF.Square)
                nc.tensor.matmul(x2_ps[:], ones_f_col[:, :], sq[:],
                                 start=(do == 0), stop=(do == ND - 1))
            # scalars: mu, var, rstd  (all [1, QS])
            mu = small.tile([1, QS], F32, tag="mu")
            m2 = small.tile([1, QS], F32, tag="m2")
            var = small.tile([1, QS], F32, tag="var")
            rvar = small.tile([1, QS], F32, tag="rvar")
            rstd = small.tile([1, QS], F32, tag="rstd")
            nc.vector.tensor_scalar_mul(mu[:], mu_ps[:], 1.0 / D)
            nc.vector.tensor_scalar_mul(m2[:], x2_ps[:], 1.0 / D)
            nc.vector.tensor_tensor(var[:], mu[:], mu[:], ALU.mult)
            nc.vector.scalar_tensor_tensor(var[:], m2[:], LN_EPS, var[:],
                                           ALU.add, ALU.subtract)
            nc.vector.reciprocal(rvar[:], var[:])
            nc.scalar.sqrt(rstd[:], rvar[:])
            # broadcast mu and rstd to [P, QS]
            bmu_ps = ps_o.tile([P, QS], F32, tag="pso")
            nc.tensor.matmul(bmu_ps[:], ones_f_row[:, :], mu[:],
                             start=True, stop=True)
            brs_ps = ps_o.tile([P, QS], F32, tag="pso")
            nc.tensor.matmul(brs_ps[:], ones_f_row[:, :], rstd[:],
                             start=True, stop=True)
            bmu = small.tile([P, QS], F32, tag="bmu")
            brs = small.tile([P, QS], F32, tag="brs")
            nc.vector.tensor_copy(bmu[:], bmu_ps[:])
            nc.vector.tensor_copy(brs[:], brs_ps[:])
            yT_r = yT.rearrange("(c p) q -> p c q", p=P)
            for do in range(ND):
                t = lnt.tile([P, QS], F32, tag="t")
                ysb = lnt.tile([P, QS], F32, tag="ysb")
                nc.vector.tensor_sub(t[:], resid[:, do, :], bmu[:])
                nc.vector.tensor_mul(t[:], t[:], brs[:])
                nc.vector.tensor_scalar(
                    ysb[:], t[:],
                    gamma_sb[:, do:do + 1], beta_sb[:, do:do + 1],
                    ALU.mult, ALU.add,
                )
                nc.sync.dma_start(yT_r[:, do, :], ysb[:])

        invp.release()
        bcp.release()
        small.release()
        persist.release()

    nc.compile()
    return nc


_NC_CACHE = None


def _get_program():
    global _NC_CACHE
    if _NC_CACHE is None:
        _NC_CACHE = build_program()
    return _NC_CACHE


def make_in_maps(features, locations, Wloc, bloc, Wq, bq, Wk, bk, Wv, bv,
                 Wb, bb, Wo, bo, gamma, beta):
    bf = ml_dtypes.bfloat16
    wlocTe = np.concatenate([Wloc.T, bloc[None, :]], axis=0).astype(bf)
    wqT = Wq.T.astype(bf)
    wkT = Wk.T.astype(bf)
    wvT = Wv.T.astype(bf)
    wo8T = (8.0 * Wo).T.astype(bf)
    bq_pc = np.ascontiguousarray(bq.reshape(ND, P).T.astype(np.float32))
    bk_pc = np.ascontiguousarray(bk.reshape(ND, P).T.astype(np.float32))
    bv_row = bv[None, :].astype(bf)
    bo_pc = np.ascontiguousarray(bo.reshape(ND, P).T.astype(np.float32))
    gamma_pc = np.ascontiguousarray(gamma.reshape(ND, P).T.astype(np.float32))
    beta_pc = np.ascontiguousarray(beta.reshape(ND, P).T.astype(np.float32))
    in_maps = []
    for c in range(8):
        b = c // 4
        qoff = (c % 4) * QS
        featq = np.ascontiguousarray(features[b][qoff:qoff + QS].T.astype(np.float32))
        locTe = np.concatenate(
            [locations[b][qoff:qoff + QS].T, np.ones((1, QS), np.float32)],
            axis=0).astype(bf)
        in_maps.append({
            "featq": featq, "locTe": locTe,
            "wlocTe": wlocTe, "wqT": wqT, "wkT": wkT, "wvT": wvT,
            "wo8T": wo8T, "bq_pc": bq_pc, "bk_pc": bk_pc, "bv_row": bv_row,
            "bo_pc": bo_pc, "gamma_pc": gamma_pc, "beta_pc": beta_pc,
        })
    return in_maps


def unshard(results):
    y = np.empty((B, S, D), np.float32)
    avg = np.empty((B, S, S), np.float32)
    for c in range(8):
        b = c // 4
        qoff = (c % 4) * QS
        yT_out = np.asarray(results[c]["yT"], dtype=np.float32)       # (D, QS)
        avgT_out = np.asarray(results[c]["avgT"]).astype(np.float32)  # (S, QS)
        y[b, qoff:qoff + QS, :] = yT_out.T
        avg[b, qoff:qoff + QS, :] = avgT_out.T
    return y, avg


def kernel(**inputs):
    inputs = {k: np.asarray(v) for k, v in inputs.items()}
    nc = _get_program()
    in_maps = make_in_maps(**inputs)
    res = run_bass_kernel_spmd(nc, in_maps, core_ids=list(range(8)))
    return unshard(res.results)


# revision 21
# speedup vs baseline: 134.9622x; 74.9640x over previous
"""Trainium2 Bass kernel for AIFSLocationAwareAttention (B=2,S=2048,D=1024,L=64,H=8).

Sharding: 8 cores; core c handles batch c//4, query rows [qoff, qoff+512),
all 8 heads locally.  K/V are computed on the owning core's query shard and
AllGathered within the 4-core batch group, so every projection FLOP is done
exactly once.  Outputs are per-core disjoint slices, reassembled on host.

Math notes:
 - loc_bias (Wb/bb) adds a per-query constant along the softmax axis -> no-op
   under softmax; skipped entirely.
 - softmax computed without max subtraction (logits are ~N(0,0.4), safe).
 - per-head normalization folded as 1/(8*s); Wo pre-scaled by 8 on host, so
   one scaled exp() tensor serves both the attended matmul and the
   mean-over-heads output.
 - all matmul operands bf16 (PSUM accumulation stays f32).
"""

import sys

sys.path.insert(0, "/opt/trn_rl_repo")

import numpy as np
import ml_dtypes

import concourse.bass as bass  # noqa: F401
import concourse.tile as tile
from concourse import bacc, mybir
from concourse.bass_utils import run_bass_kernel_spmd

P = 128
B, S, D, L, H = 2, 2048, 1024, 64, 8
DH = D // H           # 128
QS = 512              # queries per core
NKC = S // P          # 16 key chunks
ND = D // P           # 8 feature chunks
NQC = QS // P         # 4 own key chunks
GW = 4                # allgather group width
LN_EPS = 1e-5
SCALE = 1.0 / np.sqrt(np.float32(DH))

F32 = mybir.dt.float32
BF16 = mybir.dt.bfloat16
AF = mybir.ActivationFunctionType
ALU = mybir.AluOpType


def build_program(reps=1):
    nc = bacc.Bacc(None, target_bir_lowering=False)

    # ---- per-core DRAM inputs (uniform shapes; data differs per core) ----
    featq = nc.dram_tensor("featq", [D, QS], F32, kind="ExternalInput")
    locTe = nc.dram_tensor("locTe", [L + 1, QS], BF16, kind="ExternalInput")
    wlocTe = nc.dram_tensor("wlocTe", [L + 1, D], BF16, kind="ExternalInput")
    wqT = nc.dram_tensor("wqT", [D, D], BF16, kind="ExternalInput")
    wkT = nc.dram_tensor("wkT", [D, D], BF16, kind="ExternalInput")
    wvT = nc.dram_tensor("wvT", [D, D], BF16, kind="ExternalInput")
    wo8T = nc.dram_tensor("wo8T", [D, D], BF16, kind="ExternalInput")
    bq_pc = nc.dram_tensor("bq_pc", [P, ND], F32, kind="ExternalInput")
    bk_pc = nc.dram_tensor("bk_pc", [P, ND], F32, kind="ExternalInput")
    bv_row = nc.dram_tensor("bv_row", [1, D], BF16, kind="ExternalInput")
    bo_pc = nc.dram_tensor("bo_pc", [P, ND], F32, kind="ExternalInput")
    gamma_pc = nc.dram_tensor("gamma_pc", [P, ND], F32, kind="ExternalInput")
    beta_pc = nc.dram_tensor("beta_pc", [P, ND], F32, kind="ExternalInput")

    yT = nc.dram_tensor("yT", [D, QS], F32, kind="ExternalOutput")
    avgT = nc.dram_tensor("avgT", [S, QS], BF16, kind="ExternalOutput")

    # K/V allgather bounce buffers (internal DRAM)
    # block 0: K shard as (D, QS) row-major; block 1: V shard as (QS, D)
    kvag_in = nc.dram_tensor("kvag_in", [2, D * QS], BF16)
    kvag_out = nc.dram_tensor("kvag_out", [GW, 2, D * QS], BF16)

    with tile.TileContext(nc) as tc:
      for _rep in range(reps):
        # ---------- persistent SBUF ----------
        persist = tc.alloc_tile_pool(name="persist", bufs=1)
        KT_sb = persist.tile([P, ND, S], BF16, tag="KT")        # [dh, h, k]
        V_sb = persist.tile([P, NKC, D], BF16, tag="V")         # [k_in, kc, d]
        QT_sb = persist.tile([P, ND, QS], BF16, tag="QT")       # [dh, h, q]
        attn_sb = persist.tile([P, ND, QS], BF16, tag="attn")   # [dh, h, q]
        acc_sb = persist.tile([P, NKC, QS], BF16, tag="acc")    # avg accum
        fq_sb = persist.tile([P, ND, QS], F32, tag="fq")        # features (resid)

        small = tc.alloc_tile_pool(name="small", bufs=1)
        bcp = tc.alloc_tile_pool(name="bcast", bufs=2)
        invp = tc.alloc_tile_pool(name="invs", bufs=2)
        bq_sb = small.tile([P, ND], F32, tag="bq")
        bk_sb = small.tile([P, ND], F32, tag="bk")
        bo_sb = small.tile([P, ND], F32, tag="bo")
        gamma_sb = small.tile([P, ND], F32, tag="gamma")
        beta_sb = small.tile([P, ND], F32, tag="beta")
        bv_sb = small.tile([1, D], BF16, tag="bv")
        ones_bf_col = small.tile([P, 1], BF16, tag="ones_bf")    # lhsT for key-sums
        ones_bf_row = small.tile([1, P], BF16, tag="onesr_bf")   # lhsT for V bias
        ones_f_col = small.tile([P, 1], F32, tag="ones_f")       # lhsT for LN sums
        ones_f_row = small.tile([1, P], F32, tag="onesr_f")      # lhsT for broadcasts

        nc.sync.dma_start(bq_sb[:], bq_pc[:])
        nc.sync.dma_start(bk_sb[:], bk_pc[:])
        nc.sync.dma_start(bo_sb[:], bo_pc[:])
        nc.sync.dma_start(gamma_sb[:], gamma_pc[:])
        nc.sync.dma_start(beta_sb[:], beta_pc[:])
        nc.sync.dma_start(bv_sb[:], bv_row[:])
        nc.sync.dma_start(fq_sb[:], featq.rearrange("(c p) q -> p c q", p=P))
        nc.vector.memset(ones_bf_col[:], 1.0)
        nc.vector.memset(ones_bf_row[:], 1.0)
        nc.vector.memset(ones_f_col[:], 1.0)
        nc.vector.memset(ones_f_row[:], 1.0)

        # ---------- phase 1+2: combined; K/V shards; allgather; Q ----------
        with (
            tc.tile_pool(name="p12", bufs=1) as p12,
            tc.tile_pool(name="wx", bufs=2) as wxp,
            tc.tile_pool(name="psum12", bufs=3, space="PSUM") as ps12,
        ):
            locTe_sb = p12.tile([L + 1, QS], BF16, tag="locTe")
            wlocTe_sb = p12.tile([L + 1, D], BF16, tag="wlocTe")
            combT = p12.tile([P, ND, QS], BF16, tag="combT")
            nc.sync.dma_start(locTe_sb[:], locTe[:])
            nc.sync.dma_start(wlocTe_sb[:], wlocTe[:])

            for di in range(ND):
                ps = ps12.tile([P, D], F32, tag="ps_big")
                nc.tensor.matmul(
                    ps[:, :QS],
                    wlocTe_sb[:, di * P:(di + 1) * P],
                    locTe_sb[:, :],
                    start=True, stop=True,
                )
                nc.vector.tensor_add(combT[:, di, :], ps[:, :QS], fq_sb[:, di, :])

            # --- K shard: KTs[dh, h, own-q] (+bk via ACT bias) ---
            KTs = p12.tile([P, ND, QS], BF16, tag="KTs")
            wk_sb = wxp.tile([P, ND, D], BF16, tag="wx")
            nc.sync.dma_start(wk_sb[:], wkT.rearrange("(s p) d -> p s d", p=P))
            for do in range(ND):
                ps = ps12.tile([P, D], F32, tag="ps_big")
                for st in range(ND):
                    nc.tensor.matmul(
                        ps[:, :QS],
                        wk_sb[:, st, do * P:(do + 1) * P],
                        combT[:, st, :],
                        start=(st == 0), stop=(st == ND - 1),
                    )
                nc.scalar.activation(KTs[:, do, :], ps[:, :QS], AF.Identity,
                                     bias=bk_sb[:, do:do + 1], scale=1.0)
            nc.sync.dma_start(
                kvag_in[0].rearrange("(c p q) -> p c q", p=P, q=QS), KTs[:])

            # --- V shard: Vs[k_in, own-kc, d] (+bv via ones-row matmul) ---
            Vs = p12.tile([P, NQC, D], BF16, tag="Vs")
            wv_sb = wxp.tile([P, ND, D], BF16, tag="wx")
            nc.sync.dma_start(wv_sb[:], wvT.rearrange("(s p) d -> p s d", p=P))
            for kc in range(NQC):
                ps = ps12.tile([P, D], F32, tag="ps_big")
                for st in range(ND):
                    for nb in range(D // 512):
                        nc.tensor.matmul(
                            ps[:, nb * 512:(nb + 1) * 512],
                            combT[:, st, kc * P:(kc + 1) * P],
                            wv_sb[:, st, nb * 512:(nb + 1) * 512],
                            start=(st == 0), stop=False,
                        )
                for nb in range(D // 512):
                    nc.tensor.matmul(
                        ps[:, nb * 512:(nb + 1) * 512],
                        ones_bf_row[:, :],
                        bv_sb[:, nb * 512:(nb + 1) * 512],
                        start=False, stop=True,
                    )
                nc.vector.tensor_copy(Vs[:, kc, :], ps[:])
            nc.sync.dma_start(
                kvag_in[1].rearrange("(c p d) -> p c d", p=P, d=D), Vs[:])

            # --- allgather K|V within the batch group ---
            nc.gpsimd.collective_compute(
                "AllGather",
                ALU.bypass,
                ins=[kvag_in[:]],
                outs=[kvag_out[:]],
                replica_groups=[[0, 1, 2, 3], [4, 5, 6, 7]],
            )

            # --- Q projection (overlaps the allgather) ---
            wq_sb = wxp.tile([P, ND, D], BF16, tag="wx")
            nc.sync.dma_start(wq_sb[:], wqT.rearrange("(s p) d -> p s d", p=P))
            for do in range(ND):
                ps = ps12.tile([P, D], F32, tag="ps_big")
                for st in range(ND):
                    nc.tensor.matmul(
                        ps[:, :QS],
                        wq_sb[:, st, do * P:(do + 1) * P],
                        combT[:, st, :],
                        start=(st == 0), stop=(st == ND - 1),
                    )
                nc.scalar.activation(QT_sb[:, do, :], ps[:, :QS], AF.Identity,
                                     bias=bq_sb[:, do:do + 1], scale=1.0)

            # --- unpack gathered K/V into full-sequence SBUF tiles ---
            agK = kvag_out[:, 0].rearrange("r (c p q) -> r c p q", p=P, q=QS)
            for h in range(ND):
                nc.sync.dma_start(
                    KT_sb[:, h, :].rearrange("p (r q) -> p r q", r=GW),
                    agK[:, h].rearrange("r p q -> p r q"),
                )
            for r in range(GW):
                nc.sync.dma_start(
                    V_sb[:, r * NQC:(r + 1) * NQC, :],
                    kvag_out[r, 1].rearrange("(k p d) -> p k d", p=P, d=D),
                )

        # ---------- phase 3: attention per head ----------
        with (
            tc.tile_pool(name="ew", bufs=2) as ewp,
            tc.tile_pool(name="ps_sc", bufs=2, space="PSUM") as ps_sc,
            tc.tile_pool(name="ps_att", bufs=2, space="PSUM") as ps_att,
            tc.tile_pool(name="ps_sum", bufs=1, space="PSUM") as ps_sum,
        ):
            for h in range(H):
                ew = ewp.tile([P, NKC, QS], BF16, tag="ew")
                sums = ps_sum.tile([1, QS], F32, tag="sums")
                att = ps_att.tile([P, QS], F32, tag="attps")
                for sg in range(NKC // 2):
                    sc = ps_sc.tile([P, 2, QS], F32, tag="sc")
                    for j in range(2):
                        c = 2 * sg + j
                        nc.tensor.matmul(
                            sc[:, j, :],
                            KT_sb[:, h, c * P:(c + 1) * P],
                            QT_sb[:, h, :],
                            start=True, stop=True,
                        )
                    nc.scalar.activation(ew[:, 2 * sg:2 * sg + 2, :], sc[:],
                                         AF.Exp, scale=float(SCALE))
                    for j in range(2):
                        c = 2 * sg + j
                        nc.tensor.matmul(
                            sums[:, :], ones_bf_col[:, :], ew[:, c, :],
                            start=(c == 0), stop=(c == NKC - 1),
                        )
                        nc.tensor.matmul(
                            att[:, :], V_sb[:, c, h * P:(h + 1) * P], ew[:, c, :],
                            start=(c == 0), stop=(c == NKC - 1),
                        )
                # inv_s broadcast over partitions via PE, then *1/8 -> bf16
                inv_s = invp.tile([1, QS], F32, tag="invs")
                nc.vector.reciprocal(inv_s[:], sums[:])
                bc_ps = ps_att.tile([P, QS], F32, tag="attps")
                nc.tensor.matmul(bc_ps[:], ones_f_row[:, :], inv_s[:],
                                 start=True, stop=True)
                bc8 = bcp.tile([P, QS], BF16, tag="bc8")
                nc.vector.tensor_scalar_mul(bc8[:], bc_ps[:], 0.125)
                # attended (normalized by 1/(8s); Wo is pre-scaled by 8)
                nc.vector.tensor_mul(attn_sb[:, h, :], att[:], bc8[:])
                # weights*(1/(8s)) accumulated into the head-mean
                nc.vector.tensor_tensor(
                    ew[:], ew[:],
                    bc8[:, None, :].to_broadcast([P, NKC, QS]),
                    ALU.mult,
                )
                if h == 0:
                    nc.vector.tensor_copy(acc_sb[:], ew[:])
                else:
                    nc.vector.tensor_add(acc_sb[:], acc_sb[:], ew[:])

        nc.sync.dma_start(avgT.rearrange("(c p) q -> p c q", p=P), acc_sb[:])

        # ---------- phase 4: out-proj + residual + LayerNorm ----------
        with (
            tc.tile_pool(name="ln", bufs=1) as lnp,
            tc.tile_pool(name="lnt", bufs=3) as lnt,
            tc.tile_pool(name="wo", bufs=1) as wop,
            tc.tile_pool(name="ps_o", bufs=3, space="PSUM") as ps_o,
            tc.tile_pool(name="ps_s", bufs=2, space="PSUM") as ps_s,
        ):
            wo_sb = wop.tile([P, ND, D], BF16, tag="wo")
            nc.sync.dma_start(wo_sb[:], wo8T.rearrange("(s p) d -> p s d", p=P))
            resid = lnp.tile([P, ND, QS], F32, tag="resid")
            mu_ps = ps_s.tile([1, QS], F32, tag="stat")
            x2_ps = ps_s.tile([1, QS], F32, tag="stat")
            for do in range(ND):
                ps = ps_o.tile([P, QS], F32, tag="pso")
                for st in range(ND):
                    nc.tensor.matmul(
                        ps[:, :],
                        wo_sb[:, st, do * P:(do + 1) * P],
                        attn_sb[:, st, :],
                        start=(st == 0), stop=(st == ND - 1),
                    )
                # resid = (ps + bo) + featq
                nc.vector.scalar_tensor_tensor(
                    resid[:, do, :], ps[:], bo_sb[:, do:do + 1], fq_sb[:, do, :],
                    ALU.add, ALU.add,
                )
                nc.tensor.matmul(mu_ps[:], ones_f_col[:, :], resid[:, do, :],
                                 start=(do == 0), stop=(do == ND - 1))
                sq = lnt.tile([P, QS], F32, tag="sq")
                nc.scalar.activation(sq[:], resid[:, do, :], AF.Square)
                nc.tensor.matmul(x2_ps[:], ones_f_col[:, :], sq[:],
                                 start=(do == 0), stop=(do == ND - 1))
            # scalars: mu, var, rstd  (all [1, QS])
            mu = small.tile([1, QS], F32, tag="mu")
            m2 = small.tile([1, QS], F32, tag="m2")
            var = small.tile([1, QS], F32, tag="var")
            rvar = small.tile([1, QS], F32, tag="rvar")
            rstd = small.tile([1, QS], F32, tag="rstd")
            nc.vector.tensor_scalar_mul(mu[:], mu_ps[:], 1.0 / D)
            nc.vector.tensor_scalar_mul(m2[:], x2_ps[:], 1.0 / D)
            nc.vector.tensor_tensor(var[:], mu[:], mu[:], ALU.mult)
            nc.vector.scalar_tensor_tensor(var[:], m2[:], LN_EPS, var[:],
                                           ALU.add, ALU.subtract)
            nc.vector.reciprocal(rvar[:], var[:])
            nc.scalar.sqrt(rstd[:], rvar[:])
            # broadcast mu and rstd to [P, QS]
            bmu_ps = ps_o.tile([P, QS], F32, tag="pso")
            nc.tensor.matmul(bmu_ps[:], ones_f_row[:, :], mu[:],
                             start=True, stop=True)
            brs_ps = ps_o.tile([P, QS], F32, tag="pso")
            nc.tensor.matmul(brs_ps[:], ones_f_row[:, :], rstd[:],
                             start=True, stop=True)
            bmu = small.tile([P, QS], F32, tag="bmu")
            brs = small.tile([P, QS], F32, tag="brs")
            nc.vector.tensor_copy(bmu[:], bmu_ps[:])
            nc.vector.tensor_copy(brs[:], brs_ps[:])
            yT_r = yT.rearrange("(c p) q -> p c q", p=P)
            for do in range(ND):
                t = lnt.tile([P, QS], F32, tag="t")
                ysb = lnt.tile([P, QS], F32, tag="ysb")
                nc.vector.tensor_sub(t[:], resid[:, do, :], bmu[:])
                nc.vector.tensor_mul(t[:], t[:], brs[:])
                nc.vector.tensor_scalar(
                    ysb[:], t[:],
                    gamma_sb[:, do:do + 1], beta_sb[:, do:do + 1],
                    ALU.mult, ALU.add,
                )
                nc.sync.dma_start(yT_r[:, do, :], ysb[:])

        invp.release()
        bcp.release()
        small.release()
        persist.release()

    nc.compile()
    return nc


_NC_CACHE = None


def _get_program():
    global _NC_CACHE
    if _NC_CACHE is None:
        _NC_CACHE = build_program()
    return _NC_CACHE


def make_in_maps(features, locations, Wloc, bloc, Wq, bq, Wk, bk, Wv, bv,
                 Wb, bb, Wo, bo, gamma, beta):
    bf = ml_dtypes.bfloat16
    wlocTe = np.concatenate([Wloc.T, bloc[None, :]], axis=0).astype(bf)
    wqT = Wq.T.astype(bf)
    wkT = Wk.T.astype(bf)
    wvT = Wv.T.astype(bf)
    wo8T = (8.0 * Wo).T.astype(bf)
    bq_pc = np.ascontiguousarray(bq.reshape(ND, P).T.astype(np.float32))
    bk_pc = np.ascontiguousarray(bk.reshape(ND, P).T.astype(np.float32))
    bv_row = bv[None, :].astype(bf)
    bo_pc = np.ascontiguousarray(bo.reshape(ND, P).T.astype(np.float32))
    gamma_pc = np.ascontiguousarray(gamma.reshape(ND, P).T.astype(np.float32))
    beta_pc = np.ascontiguousarray(beta.reshape(ND, P).T.astype(np.float32))
    in_maps = []
    for c in range(8):
        b = c // 4
        qoff = (c % 4) * QS
        featq = np.ascontiguousarray(features[b][qoff:qoff + QS].T.astype(np.float32))
        locTe = np.concatenate(
            [locations[b][qoff:qoff + QS].T, np.ones((1, QS), np.float32)],
            axis=0).astype(bf)
        in_maps.append({
            "featq": featq, "locTe": locTe,
            "wlocTe": wlocTe, "wqT": wqT, "wkT": wkT, "wvT": wvT,
            "wo8T": wo8T, "bq_pc": bq_pc, "bk_pc": bk_pc, "bv_row": bv_row,
            "bo_pc": bo_pc, "gamma_pc": gamma_pc, "beta_pc": beta_pc,
        })
    return in_maps


def unshard(results):
    y = np.empty((B, S, D), np.float32)
    avg = np.empty((B, S, S), np.float32)
    for c in range(8):
        b = c // 4
        qoff = (c % 4) * QS
        yT_out = np.asarray(results[c]["yT"], dtype=np.float32)       # (D, QS)
        avgT_out = np.asarray(results[c]["avgT"]).astype(np.float32)  # (S, QS)
        y[b, qoff:qoff + QS, :] = yT_out.T
        avg[b, qoff:qoff + QS, :] = avgT_out.T
    return y, avg


def kernel(**inputs):
    inputs = {k: np.asarray(v) for k, v in inputs.items()}
    nc = _get_program()
    in_maps = make_in_maps(**inputs)
    res = run_bass_kernel_spmd(nc, in_maps, core_ids=list(range(8)))
    return unshard(res.results)
